# revision 1
# baseline (speedup 1.0000x reference)
"""Block-causal self-attention TRN2 kernel.

Sharding: the 64 (batch x block) units are independent -> 8 per NeuronCore,
zero cross-core traffic. Full inputs in, full output out.

Per-core strategy (all matmuls float32r = full-rate 4-byte):
- x loaded naturally, transposed on the PE (identity matmul) -> xT tiles.
- qkT[d, t] = w_qk.T @ xT (features on partitions).
- RMSNorm over d (=partitions): squares on ACT from PSUM, column sums via
  ones-matmul, rsqrt applied as (ln per-partition scalar) x (r broadcast via
  rank-1 ones outer-product), fused in one scalar_tensor_tensor.
- scoresT[tk, tq] = kT_h.T @ qT_h, two heads packed per [128, 512] tile
  (softmax over PARTITION dim tk): exp on ACT (scores bounded, no max
  subtraction), causal mask via gpsimd affine_select (fill=0, 2D pattern).
- v in natural layout [t, d] with a ones-column appended per head (65-wide):
  the av matmul then yields the softmax denominators for free in psum row 64.
- yT[d, tq] per head: lhsT=v_aug[tk, head], rhs=masked expT; normalization
  (1/sums) folded into the yT PSUM evacuation via a rank-1 broadcast.
- out[t, e] = yT.T @ w_proj -> natural output layout, direct DMA out.
- proj for t-half 0 is split per d-tile and interleaved into the attention
  pairs so the PE stays fed while exp/mask round-trip through ACT/GPSIMD.
"""

import numpy as np

import concourse.bacc as bacc
import concourse.tile as tile
from concourse import mybir
from concourse.bass_utils import run_bass_kernel_spmd

F32 = mybir.dt.float32
F32R = mybir.dt.float32r

B, T, C = 4, 4096, 1024
H, HD, LS = 16, 64, 256
NCORES = 8
NBLK = (B * T) // LS  # 64
BPC = NBLK // NCORES  # 8 blocks per core
TOK = BPC * LS  # 2048 tokens per core
CT = C // 128  # 8 c-tiles
EPS = 1e-6
VW = HD + 1  # 65: per-head v width incl ones column

AF = mybir.ActivationFunctionType
OP = mybir.AluOpType


def build(variant=8):
    # variant bisect: 1=front only, 2=+attention (simple proj), 3=full
    nc = bacc.Bacc()
    x = nc.declare_dram_parameter("x", [TOK, C], F32, isOutput=False)
    w_qkv = nc.declare_dram_parameter("w_qkv", [C, 3 * C], F32, isOutput=False)
    ln_w = nc.declare_dram_parameter("ln_w", [C], F32, isOutput=False)
    w_proj = nc.declare_dram_parameter("w_proj", [C, C], F32, isOutput=False)
    out = nc.declare_dram_parameter("out", [TOK, C], F32, isOutput=True)

    with tile.TileContext(nc) as tc:
        with (
            tc.tile_pool(name="const", bufs=1) as cpool,
            tc.tile_pool(name="xt", bufs=2) as xt_pool,
            tc.tile_pool(name="qk", bufs=1) as qk_pool,
            tc.tile_pool(name="work", bufs=2) as work,
            tc.tile_pool(name="vy", bufs=1) as vy_pool,
            tc.tile_pool(name="ep", bufs=2) as ep_pool,
            tc.tile_pool(name="small", bufs=2) as small,
            tc.tile_pool(name="psA", bufs=3, space="PSUM") as psA,
            tc.tile_pool(name="psB", bufs=2, space="PSUM") as psB,
            tc.tile_pool(name="psH", bufs=1, space="PSUM") as psH,
            tc.tile_pool(name="pstiny", bufs=2, space="PSUM") as pstiny,
        ):
            # ---- constants (loaded once) ----
            wq_sb = cpool.tile([128, CT, 3 * C], F32R)
            for ct in range(CT):
                nc.sync.dma_start(
                    out=wq_sb[:, ct],
                    in_=w_qkv[ct * 128 : (ct + 1) * 128, :].bitcast(F32R),
                )
            wp_sb = cpool.tile([128, CT, C], F32R)
            nc.sync.dma_start(
                out=wp_sb,
                in_=w_proj.rearrange("(ct p) d -> p ct d", p=128).bitcast(F32R),
            )
            ln_sb = cpool.tile([128, CT], F32)
            nc.sync.dma_start(out=ln_sb, in_=ln_w.rearrange("(ct p) -> p ct", p=128))
            ones_stage = cpool.tile([128, 1], F32)
            nc.vector.memset(ones_stage, 1.0)
            ones_row_stage = cpool.tile([1, 128], F32)
            nc.vector.memset(ones_row_stage, 1.0)
            ones_row = cpool.tile([1, 128], F32R)
            nc.vector.tensor_copy(out=ones_row, in_=ones_row_stage)
            ones_col = cpool.tile([128, 1], F32R)
            nc.vector.tensor_copy(out=ones_col, in_=ones_stage)
            bias_q = cpool.tile([1, 1], F32)
            nc.vector.memset(bias_q, 64.0 * EPS)
            bias_k = cpool.tile([1, 1], F32)
            nc.vector.memset(bias_k, EPS)
            # identity for PE transpose
            ident_stage = cpool.tile([128, 128], F32)
            nc.vector.memset(ident_stage, 1.0)
            nc.gpsimd.affine_select(
                out=ident_stage,
                in_=ident_stage,
                pattern=[[1, 128]],
                compare_op=OP.is_equal,
                fill=0.0,
                base=0,
                channel_multiplier=-1,
            )
            ident = cpool.tile([128, 128], F32R)
            nc.vector.tensor_copy(out=ident, in_=ident_stage)
            # causal 0/1 masks per tk-tile: keep iff tq >= tk + 128*tk_t
            mask_stage_t = work.tile([128, C], F32, tag="ostage", name="mstage")
            mask_stage = mask_stage_t[:, 0 : 2 * LS]
            masks = []
            for tk_t in range(2):
                nc.vector.memset(mask_stage, 1.0)
                nc.gpsimd.affine_select(
                    out=mask_stage,
                    in_=mask_stage,
                    pattern=[[0, 2], [1, LS]],
                    compare_op=OP.is_ge,
                    fill=0.0,
                    base=-(tk_t * 128),
                    channel_multiplier=-1,
                )
                mk = cpool.tile([128, 2 * LS], F32R, name=f"mask{tk_t}")
                nc.vector.tensor_copy(out=mk, in_=mask_stage)
                masks.append(mk)
            # persistent augmented-v tile: [tk-tile][head * 65 + (hd | ones)]
            v_aug = cpool.tile([128, 2, H * VW], F32R)
            for tt in range(2):
                nc.vector.tensor_copy(
                    out=v_aug[:, tt].rearrange("p (h w) -> p h w", w=VW)[:, :, HD],
                    in_=ones_col.to_broadcast((128, H)),
                )

            for b in range(BPC):
                t0 = b * LS
                # ---- natural x load + PE transpose into xT ----
                xT = [
                    xt_pool.tile(
                        [128, LS], F32R, tag=f"xt{ct}", name=f"xt_{b}_{ct}", bufs=1
                    )
                    for ct in range(CT)
                ]
                for tt in range(2):
                    x_nat = xt_pool.tile(
                        [128, C], F32R, tag="xn", name=f"xn_{b}_{tt}", bufs=2
                    )
                    nc.scalar.dma_start(
                        out=x_nat,
                        in_=x[t0 + tt * 128 : t0 + (tt + 1) * 128, :].bitcast(F32R),
                    )
                    for ct in range(CT):
                        tp = psA.tile(
                            [128, 128], F32R, tag="psA", name=f"tp_{b}_{ct}_{tt}"
                        )
                        nc.tensor.transpose(
                            tp, x_nat[:, ct * 128 : (ct + 1) * 128], ident
                        )
                        nc.vector.tensor_copy(
                            out=xT[ct][:, tt * 128 : (tt + 1) * 128], in_=tp
                        )

                # ---- qkT d-tiles (0..7 q, 8..15 k) + squares for rmsnorm ----
                qk = []
                sq_pk = pstiny.tile([1, 2 * LS], F32, tag="tiny", name=f"sq_{b}")
                for dt in range(16):
                    half = dt // 8
                    ps = psA.tile([128, LS], F32, tag="psA", name=f"qkps_{b}_{dt}")
                    for ct in range(CT):
                        nc.tensor.matmul(
                            ps,
                            wq_sb[:, ct, dt * 128 : (dt + 1) * 128],
                            xT[ct],
                            start=(ct == 0),
                            stop=(ct == CT - 1),
                        )
                    t_ = qk_pool.tile(
                        [128, LS], F32R, tag=f"qk{dt}", name=f"qk_{b}_{dt}"
                    )
                    if dt % 4 == 3:
                        nc.scalar.activation(out=t_, in_=ps, func=AF.Copy)
                    else:
                        nc.vector.tensor_copy(out=t_, in_=ps)
                    qk.append(t_)
                    q2 = work.tile([128, LS], F32R, tag="q2", name=f"q2_{b}_{dt}")
                    nc.scalar.activation(out=q2, in_=ps, func=AF.Square)
                    nc.tensor.matmul(
                        sq_pk[:, half * LS : (half + 1) * LS],
                        ones_col,
                        q2,
                        start=(dt % 8 == 0),
                        stop=(dt % 8 == 7),
                        skip_group_check=True,
                    )

                # ---- v in natural layout, scattered per head into v_aug ----
                for tt in range(2):
                    for ch in range(2):
                        vps = psH.tile(
                            [128, 512], F32, tag="psH", name=f"vps_{b}_{tt}_{ch}"
                        )
                        for ct in range(CT):
                            nc.tensor.matmul(
                                vps,
                                xT[ct][:, tt * 128 : (tt + 1) * 128],
                                wq_sb[:, ct, 2 * C + ch * 512 : 2 * C + (ch + 1) * 512],
                                start=(ct == 0),
                                stop=(ct == CT - 1),
                            )
                        nc.vector.tensor_copy(
                            out=v_aug[:, tt, 8 * ch * VW :].rearrange(
                                "p (h w) -> p h w", w=VW
                            )[:, 0:8, 0:HD],
                            in_=vps.rearrange("p (h w) -> p h w", w=HD),
                        )

                # ---- RMSNorm apply (q: fold in 1/sqrt(hd); k: plain) ----
                for half in range(2):
                    scale = (1.0 / 16.0) if half == 0 else (1.0 / 1024.0)
                    bias = bias_q if half == 0 else bias_k
                    sq_s = small.tile(
                        [1, LS], F32, tag="sqs", name=f"sqs_{b}_{half}", bufs=1
                    )
                    nc.scalar.activation(
                        out=sq_s,
                        in_=sq_pk[:, half * LS : (half + 1) * LS],
                        func=AF.Sqrt,
                        bias=bias,
                        scale=scale,
                    )
                    r_ = small.tile([1, LS], F32R, tag="r", name=f"r_{b}_{half}")
                    with nc.allow_low_precision("f32r is rounded f32"):
                        nc.vector.reciprocal(r_, sq_s)
                    rb = psA.tile([128, LS], F32, tag="psA", name=f"rb_{b}_{half}")
                    nc.tensor.matmul(rb, ones_row, r_, start=True, stop=True)
                    for dt in range(8):
                        g = qk[half * 8 + dt]
                        nc.vector.scalar_tensor_tensor(
                            out=g,
                            in0=g,
                            scalar=ln_sb[:, dt : dt + 1],
                            in1=rb,
                            op0=OP.mult,
                            op1=OP.mult,
                        )

                if variant == 1:
                    # front-only bisect: dump v_aug to out to keep it live
                    dbg = work.tile([128, C], F32, tag="ostage", name=f"dbg_{b}")
                    nc.vector.tensor_copy(out=dbg, in_=v_aug[:, 0, 0:C].bitcast(F32))
                    nc.sync.dma_start(out=out[t0 : t0 + 128, :], in_=dbg)
                    continue
                # ---- attention pairs interleaved with proj(t-half 0) ----
                yT = [None] * 8
                pps0 = [
                    psH.tile([128, 512], F32, tag="psH", name=f"pps0_{b}_{ch}")
                    for ch in range(2)
                ]

                def pair_unit(j):
                    y_ = vy_pool.tile([128, LS], F32R, tag=f"y{j}", name=f"y_{b}_{j}")
                    yT[j] = y_
                    smp = pstiny.tile([1, 2 * LS], F32, tag="tiny", name=f"smp_{b}_{j}")
                    ems = []
                    for tk in range(2):
                        er = ep_pool.tile(
                            [128, 2 * LS], F32R, tag="eraw", name=f"er_{b}_{j}_{tk}"
                        )
                        if variant == 8:
                            for h2 in range(2):
                                po = 64 * h2
                                sc8 = psA.tile(
                                    [128, LS], F32, tag="psA",
                                    name=f"sc_{b}_{j}_{tk}_{h2}",
                                )
                                nc.tensor.matmul(
                                    sc8,
                                    qk[8 + j][po : po + 64, tk * 128 : (tk + 1) * 128],
                                    qk[j][po : po + 64, :],
                                    start=True,
                                    stop=True,
                                )
                                nc.scalar.activation(
                                    out=er[:, h2 * LS : (h2 + 1) * LS],
                                    in_=sc8,
                                    func=AF.Exp,
                                )
                        else:
                            scp = psA.tile(
                                [128, 2 * LS], F32, tag="psA", name=f"sc_{b}_{j}_{tk}"
                            )
                            for h2 in range(2):
                                po = 64 * h2
                                nc.tensor.matmul(
                                    scp[:, h2 * LS : (h2 + 1) * LS],
                                    qk[8 + j][po : po + 64, tk * 128 : (tk + 1) * 128],
                                    qk[j][po : po + 64, :],
                                    start=True,
                                    stop=True,
                                    skip_group_check=True,
                                )
                            nc.scalar.activation(out=er, in_=scp, func=AF.Exp)
                        nc.vector.tensor_mul(er, er, masks[tk])
                        nc.tensor.matmul(
                            smp[:, :],
                            ones_col,
                            er,
                            start=(tk == 0),
                            stop=(tk == 1),
                            skip_group_check=True,
                        )
                        ems.append(er)
                    av = psB.tile(
                        [128, 2 * LS], F32, tag="av", name=f"av_{b}_{j}"
                    )
                    for h2 in range(2):
                        head = 2 * j + h2
                        for tk in range(2):
                            vw = HD if variant == 4 else VW
                            nc.tensor.matmul(
                                av[0:vw, h2 * LS : (h2 + 1) * LS],
                                v_aug[:, tk, head * VW : head * VW + vw],
                                ems[tk][:, h2 * LS : (h2 + 1) * LS],
                                start=(tk == 0),
                                stop=(tk == 1),
                                skip_group_check=True,
                            )
                    rc = small.tile([1, 2 * LS], F32R, tag="rc", name=f"rc_{b}_{j}", bufs=1)
                    with nc.allow_low_precision("f32r is rounded f32"):
                        nc.vector.reciprocal(rc, smp)
                    rbp = psA.tile(
                        [128, 2 * LS], F32, tag="psA", name=f"rbp_{b}_{j}"
                    )
                    nc.tensor.matmul(
                        rbp[0:64, :], ones_row[:, :64], rc, start=True, stop=True
                    )
                    rbp_s = work.tile(
                        [64, 2 * LS], F32, tag="rbps", name=f"rbps_{b}_{j}"
                    )
                    nc.vector.tensor_copy(out=rbp_s, in_=rbp[0:64, :])
                    nc.vector.tensor_mul(
                        y_[0:64, :], av[0:64, 0:LS], rbp_s[:, 0:LS]
                    )
                    ytmp = work.tile(
                        [64, LS], F32R, tag="ytmp", name=f"ytmp_{b}_{j}"
                    )
                    nc.vector.tensor_mul(
                        ytmp, av[0:64, LS : 2 * LS], rbp_s[:, LS : 2 * LS]
                    )
                    (nc.sync if variant == 7 else nc.scalar).dma_start(
                        out=y_[64:128, :], in_=ytmp
                    )

                def proj0_sub(dt):
                    for ch in range(2):
                        nc.tensor.matmul(
                            pps0[ch],
                            yT[dt][:, 0:128],
                            wp_sb[:, dt, ch * 512 : (ch + 1) * 512],
                            start=(dt == 0),
                            stop=(dt == 7),
                            skip_group_check=True,
                        )

                # schedule: pairs with proj0 subs trailing two pairs behind
                for j in range(8):
                    pair_unit(j)
                    if variant != 2 and j >= 2:
                        proj0_sub(j - 2)
                if variant == 2:
                    for dt in range(8):
                        proj0_sub(dt)
                else:
                    proj0_sub(6)
                    proj0_sub(7)
                o0 = work.tile([128, C], F32, tag="ostage", name=f"o_{b}_0")
                for ch in range(2):
                    nc.vector.tensor_copy(
                        out=o0[:, ch * 512 : (ch + 1) * 512], in_=pps0[ch]
                    )
                nc.sync.dma_start(out=out[t0 : t0 + 128, :], in_=o0)

                # proj t-half 1
                o1 = work.tile([128, C], F32, tag="ostage", name=f"o_{b}_1")
                for ch in range(2):
                    pps = psH.tile([128, 512], F32, tag="psH", name=f"pps1_{b}_{ch}")
                    for dt in range(8):
                        nc.tensor.matmul(
                            pps,
                            yT[dt][:, 128:256],
                            wp_sb[:, dt, ch * 512 : (ch + 1) * 512],
                            start=(dt == 0),
                            stop=(dt == 7),
                        )
                    nc.vector.tensor_copy(
                        out=o1[:, ch * 512 : (ch + 1) * 512], in_=pps
                    )
                nc.sync.dma_start(out=out[t0 + 128 : t0 + 256, :], in_=o1)

    nc.finalize()
    return nc


_NC_CACHE = None


def _get_nc():
    global _NC_CACHE
    if _NC_CACHE is None:
        _NC_CACHE = build()
    return _NC_CACHE


_RUNNER_CACHE = None


def _get_runner():
    """Persistent jitted shard_map over the 8 cores (mirrors
    bass2jax.run_bass_via_pjrt but reusable across calls)."""
    global _RUNNER_CACHE
    if _RUNNER_CACHE is not None:
        return _RUNNER_CACHE
    import jax
    from jax.sharding import Mesh, PartitionSpec
    from jax.experimental.shard_map import shard_map
    from concourse import bass2jax, mybir as mb

    nc = _get_nc()
    bass2jax.install_neuronx_cc_hook()
    partition_name = nc.partition_id_tensor.name if nc.partition_id_tensor else None
    in_names, out_names, out_avals, zero_shapes = [], [], [], []
    for alloc in nc.m.functions[0].allocations:
        if not isinstance(alloc, mb.MemoryLocationSet):
            continue
        name = alloc.memorylocations[0].name
        if alloc.kind == "ExternalInput":
            if name != partition_name:
                in_names.append(name)
        elif alloc.kind == "ExternalOutput":
            out_names.append(name)
            shape = tuple(alloc.tensor_shape)
            dtype = mb.dt.np(alloc.dtype)
            out_avals.append(jax.core.ShapedArray(shape, dtype))
            zero_shapes.append((shape, dtype))
    n_params = len(in_names)
    all_in = list(in_names) + list(out_names)
    if partition_name is not None:
        all_in.append(partition_name)

    def _body(*args):
        operands = list(args)
        if partition_name is not None:
            operands.append(bass2jax.partition_id_tensor())
        outs = bass2jax._bass_exec_p.bind(
            *operands,
            out_avals=tuple(out_avals),
            in_names=tuple(all_in),
            out_names=tuple(out_names),
            lowering_input_output_aliases=(),
            sim_require_finite=True,
            sim_require_nnan=True,
            nc=nc,
        )
        return tuple(outs)

    devices = jax.devices()[:NCORES]
    mesh = Mesh(np.asarray(devices), ("core",))
    nin = n_params + len(out_names)
    fn = jax.jit(
        shard_map(
            _body,
            mesh=mesh,
            in_specs=(PartitionSpec("core"),) * nin,
            out_specs=(PartitionSpec("core"),) * len(out_names),
            check_rep=False,
        ),
        keep_unused=True,
    )
    _RUNNER_CACHE = (fn, in_names, zero_shapes)
    return _RUNNER_CACHE


def kernel(x, w_qkv, ln_w, w_proj, _trace=False):
    x = np.asarray(x, dtype=np.float32)
    w_qkv = np.asarray(w_qkv, dtype=np.float32)
    ln_w = np.asarray(ln_w, dtype=np.float32)
    w_proj = np.asarray(w_proj, dtype=np.float32)

    x2 = np.ascontiguousarray(x.reshape(B * T, C))
    in_maps = [
        {
            "x": np.ascontiguousarray(x2[i * TOK : (i + 1) * TOK]),
            "w_qkv": w_qkv,
            "ln_w": ln_w,
            "w_proj": w_proj,
        }
        for i in range(NCORES)
    ]
    if _trace:
        nc = _get_nc()
        res = run_bass_kernel_spmd(
            nc, in_maps, core_ids=list(range(NCORES)), trace=True
        )
        full = np.concatenate(
            [res.results[i]["out"] for i in range(NCORES)], axis=0
        )
        return full.reshape(B, T, C).astype(np.float32), res

    fn, in_names, zero_shapes = _get_runner()
    concat_in = [
        np.concatenate([m[name] for m in in_maps], axis=0) for name in in_names
    ]
    concat_zeros = [
        np.zeros((NCORES * shape[0], *shape[1:]), dtype)
        for shape, dtype in zero_shapes
    ]
    outs = fn(*concat_in, *concat_zeros)
    full = np.asarray(outs[0])
    return full.reshape(B, T, C).astype(np.float32)

